# revision 13
# baseline (speedup 1.0000x reference)
"""Causal self-attention (B=2, S=2048, D=2048, 16 heads, RoPE) on 8 trn2 cores.

Sharding: tensor-parallel over heads x data-parallel over batch.
Core c handles batch b = c // 4 and head-group hg = c % 4 (heads 4*hg..4*hg+3).
qkv_proj is column-sharded by head, out_proj row-sharded by head; the
AllReduce of the out_proj partials is done on the host (4 partials per batch).

Per-core device program (all matmuls bf16 with fp32 PSUM accumulation).
The PE streams bf16 at ~0.52 ns/col sustained (P0 power state), with
LDWEIGHTS fully hidden, so runtime ~= total matmul output columns.  v2
therefore (a) trims the causal diagonal: S / exp / L / PV on query block
ib only compute columns [off:512] for diagonal-crossing key tiles
(off = 128*jt - 512*ib), saving ~37k of 770k columns; and (b) emits one
globally interleaved PE stream so exp (ACT) and rope (DVE) latency hides
behind independent matmuls:

  round 0: qkv token blocks 0,1 (dense)
  round 1: attn(ib0) interleaved with qkv block 2
  round 2: attn(ib1) interleaved with qkv block 3 + outproj(ib0)
  round 3: attn(ib2) interleaved with outproj(ib1)
  round 4: attn(ib3) interleaved with outproj(ib2)
  round 5: outproj(ib3)

attn runs heads in interleaved pairs (st x4 + o x2 + l x2 = 8 PSUM banks);
qkv/outproj sub-thunks of ~4 matmuls give ~1us interleave granularity.
"""

import contextlib
import math
import os

import numpy as np
import ml_dtypes

import bass_rust
import concourse.bass as bass
import concourse.mybir as mybir
import concourse.tile as tile
from concourse.bass import ts
from concourse.bass_utils import run_bass_kernel_spmd

BF16 = ml_dtypes.bfloat16
F32 = mybir.dt.float32
BF = mybir.dt.bfloat16

B = 2
S = 2048
D = 2048
HD = 128                    # head dim
NH = 16                     # total heads
NHL = 4                     # heads per core
FQ = NHL * HD               # 512 per-core q/k/v features
KC = D // 128               # 16 contraction chunks
TB = 4                      # token blocks of 512 (qkv phase)
IB = 4                      # query blocks of 512 (attention phase)
JT = S // 128               # 16 key tiles of 128
SCALE = 1.0 / math.sqrt(HD)
NEG = -30000.0              # additive mask; exp(NEG * SCALE) == 0 in fp32

MAX_WAITS = 1               # this walrus build allows 1 sync-wait per inst

_wait_ctr = [0]


def _split_all_multi_waits(nc):
    """This walrus build rejects instructions with >1 semaphore wait
    ("Too many sync wait commands").  Move extra waits onto NoOps inserted
    right before the instruction on the same engine (sequencers execute in
    order, so blocking one instruction earlier is equivalent)."""
    n_split = 0
    for f in nc.m.functions:
        for blk in f.blocks:
            out = []
            for inst in blk.instructions:
                si = inst.sync_info
                if si is not None and len(si.on_wait) > MAX_WAITS:
                    waits = list(si.on_wait)
                    for w in waits[:-MAX_WAITS]:
                        _wait_ctr[0] += 1
                        nop = mybir.InstNoOp(
                            name=f"I-waitsplit-{_wait_ctr[0]}", ins=[], outs=[]
                        )
                        nop.engine = inst.engine
                        nop.sync_info = bass_rust.SyncInfo(on_wait=[w], on_update=[])
                        out.append(nop)
                    inst.sync_info = bass_rust.SyncInfo(
                        on_wait=waits[-MAX_WAITS:], on_update=list(si.on_update)
                    )
                    n_split += 1
                out.append(inst)
            blk.instructions = out
    return n_split


def build_nc(
    reps: int = 1,
    split_waits: bool = True,
    loop: int = 1,
    do_qkv: bool = True,
    do_attn: bool = True,
    do_outproj: bool = True,
    dma_only: bool = False,
    skip_outdma: bool = False,
):
    nc = bass.Bass()
    xT = nc.declare_dram_parameter("xT", [128, TB, KC, 512], BF, isOutput=False)
    wqT = nc.declare_dram_parameter("wqT", [128, KC, FQ], BF, isOutput=False)
    wkT = nc.declare_dram_parameter("wkT", [128, KC, FQ], BF, isOutput=False)
    wvT = nc.declare_dram_parameter("wvT", [128, KC, FQ], BF, isOutput=False)
    woT = nc.declare_dram_parameter("woT", [128, NHL, D], BF, isOutput=False)
    cosT = nc.declare_dram_parameter("cosT", [128, S], BF, isOutput=False)
    sinT = nc.declare_dram_parameter("sinT", [128, S], BF, isOutput=False)
    maskd = nc.declare_dram_parameter("maskd", [128, 128], F32, isOutput=False)
    out = nc.declare_dram_parameter("out", [JT, 128, D], BF, isOutput=True)

    mult = mybir.AluOpType.mult
    add = mybir.AluOpType.add
    EXP = mybir.ActivationFunctionType.Exp

    with tile.TileContext(nc) as tc:
        with (
            tc.tile_pool(name="persist", bufs=1) as persist,
            tc.tile_pool(name="stream", bufs=1) as stream,
            tc.tile_pool(name="pp", bufs=4, space="PSUM") as pp,
        ):
            qT = persist.tile([128, NHL, S], BF, tag="qT")
            kT = persist.tile([128, NHL, S], BF, tag="kT")
            vv = persist.tile([128, JT, FQ], BF, tag="vv")   # [t-part, tt, d]
            ctx = persist.tile([128, NHL, S], BF, tag="ctx")
            cos_sb = persist.tile([128, S], BF, tag="cos")
            sin_sb = persist.tile([128, S], BF, tag="sin")
            # three persistent x-block buffers: A=block0, B=block1,
            # C=blocks 2 then 3 (reloaded mid-iteration)
            xs_bufs = [
                persist.tile(
                    [128, KC, 512], BF, tag=f"xs{i}", name=f"xs{i}"
                )
                for i in range(3)
            ]
            mask_sb = persist.tile([128, 128], F32, tag="mask")
            ones_sb = persist.tile([128, 128], BF, tag="ones")
            wq_sb = persist.tile([128, KC, FQ], BF, tag="wq")
            wk_sb = persist.tile([128, KC, FQ], BF, tag="wk")
            wv_sb = persist.tile([128, KC, FQ], BF, tag="wv")
            wo_sb = persist.tile([128, NHL, D], BF, tag="wo")

            nc.sync.dma_start(cos_sb[:], cosT[:])
            nc.sync.dma_start(sin_sb[:], sinT[:])
            nc.sync.dma_start(mask_sb[:], maskd[:])
            nc.vector.memset(ones_sb[:], 1.0)
            nc.sync.dma_start(wq_sb[:], wqT[:])
            nc.sync.dma_start(wk_sb[:], wkT[:])
            nc.sync.dma_start(wv_sb[:], wvT[:])
            nc.sync.dma_start(wo_sb[:], woT[:])

            def stage_tile():
                return stream.tile(
                    [128, D], BF, tag="stage", bufs=2, name="stage"
                )

            def qkv_thunks(tb, xs):
                """Fine-grained thunks for token block tb: 12 chains
                (4 v + 4 q + 4 k), each split into 4 sub-thunks of 4
                accumulating matmuls plus an evacuation thunk."""
                tbs = ts(tb, 512)
                thunks = []

                def qk_chain(w_sb, dstT, f):
                    box = {}

                    def mm4(g):
                        def run():
                            if g == 0:
                                box["ps"] = pp.tile(
                                    [128, 512], F32, tag="st", name="ps"
                                )
                            ps = box["ps"]
                            for kc in range(4 * g, 4 * g + 4):
                                nc.tensor.matmul(
                                    ps[:],
                                    w_sb[:, kc, ts(f, 128)],
                                    xs[:, kc, :],
                                    start=(kc == 0),
                                    stop=(kc == KC - 1),
                                )
                        return run

                    def evac():
                        # rope: dst = ps*cos + swap(ps)*sin_signed
                        ps = box["ps"]
                        t1 = stream.tile([128, 512], BF, tag="t1", bufs=2)
                        nc.vector.tensor_tensor(
                            t1[:], ps[:], cos_sb[:, tbs], mult
                        )
                        t2 = stream.tile([128, 512], BF, tag="t2", bufs=2)
                        nc.vector.tensor_tensor(
                            t2[0:64, :], ps[64:128, :], sin_sb[0:64, tbs], mult
                        )
                        nc.vector.tensor_tensor(
                            t2[64:128, :], ps[0:64, :], sin_sb[64:128, tbs],
                            mult,
                        )
                        nc.vector.tensor_tensor(
                            dstT[:, f, tbs], t1[:], t2[:], add
                        )

                    return [mm4(g) for g in range(4)] + [evac]

                def v_chain(s4):
                    box = {}

                    def mm4(g):
                        def run():
                            if g == 0:
                                box["ps"] = pp.tile(
                                    [128, 512], F32, tag="st", name="ps"
                                )
                            ps = box["ps"]
                            for kc in range(4 * g, 4 * g + 4):
                                nc.tensor.matmul(
                                    ps[:],
                                    xs[:, kc, ts(s4, 128)],
                                    wv_sb[:, kc, :],
                                    start=(kc == 0),
                                    stop=(kc == KC - 1),
                                )
                        return run

                    def evac():
                        nc.scalar.copy(vv[:, tb * 4 + s4, :], box["ps"][:])

                    return [mm4(g) for g in range(4)] + [evac]

                # k/q chains for low heads first so attention on this
                # block can start as early as possible; v interleaved
                order = [
                    ("k", 0), ("q", 0), ("k", 1), ("q", 1),
                    ("v", 0), ("v", 1),
                    ("k", 2), ("q", 2), ("k", 3), ("q", 3),
                    ("v", 2), ("v", 3),
                ]
                for kind, idx in order:
                    if kind == "v":
                        thunks.extend(v_chain(idx))
                    elif kind == "q":
                        thunks.extend(qk_chain(wq_sb, qT, idx))
                    else:
                        thunks.extend(qk_chain(wk_sb, kT, idx))
                return thunks

            def attn_thunks(ib):
                """Step-thunks: flash attention for query block ib.  Heads
                run in interleaved PAIRS so each head's exp latency hides
                behind the other head's S/L/PV matmuls.  Diagonal-crossing
                key tiles only compute columns [off:512]."""
                njt = 4 * ib + 4
                thunks = []

                def make_head(h):
                    sts = [None] * njt
                    pts = [None] * njt
                    acc = {}

                    def off_of(jt):
                        o = (jt - 4 * ib) * 128
                        return o if o > 0 else 0

                    def start_head():
                        acc["o"] = pp.tile(
                            [128, 512], F32, tag="o", bufs=2, name="o"
                        )
                        acc["l"] = pp.tile(
                            [128, 512], F32, tag="l", bufs=2, name="l"
                        )

                    def emit_s(jt):
                        off = off_of(jt)
                        st = pp.tile([128, 512], F32, tag="st", name="st")
                        nc.tensor.matmul(
                            st[:, off:512],
                            kT[:, h, ts(jt, 128)],
                            qT[:, h, ib * 512 + off : ib * 512 + 512],
                            start=True,
                            stop=True,
                        )
                        sts[jt] = st

                    def emit_exp(jt):
                        st = sts[jt]
                        off = off_of(jt)
                        pt = stream.tile([128, 512], BF, tag="pt", bufs=6)
                        if jt >= 4 * ib:
                            nc.vector.tensor_tensor(
                                st[:, off : off + 128],
                                st[:, off : off + 128],
                                mask_sb[:],
                                add,
                            )
                        nc.scalar.activation(
                            pt[:, off:512], st[:, off:512], EXP, scale=SCALE
                        )
                        pts[jt] = pt

                    def emit_l(jt):
                        off = off_of(jt)
                        nc.tensor.matmul(
                            acc["l"][:, off:512], ones_sb[:],
                            pts[jt][:, off:512],
                            start=(jt == 0), stop=(jt == njt - 1),
                        )

                    def emit_pv(jt):
                        off = off_of(jt)
                        nc.tensor.matmul(
                            acc["o"][:, off:512], vv[:, jt, ts(h, 128)],
                            pts[jt][:, off:512],
                            start=(jt == 0), stop=(jt == njt - 1),
                        )

                    def norm_head():
                        linv = stream.tile([128, 512], F32, tag="linv", bufs=2)
                        nc.vector.reciprocal(linv[:], acc["l"][:])
                        nc.vector.tensor_tensor(
                            ctx[:, h, ts(ib, 512)], acc["o"][:], linv[:], mult
                        )

                    return start_head, emit_s, emit_exp, emit_l, emit_pv, \
                        norm_head

                for hp in (0, 2):
                    s0, es0, ex0, el0, ep0, n0 = make_head(hp)
                    s1, es1, ex1, el1, ep1, n1 = make_head(hp + 1)
                    thunks.append(s0)
                    thunks.append(s1)
                    for jt in range(njt + 1):
                        if jt < njt:
                            thunks.append(lambda jt=jt, f=es0: f(jt))
                            thunks.append(lambda jt=jt, f=es1: f(jt))
                        if jt >= 1:
                            # exp for both heads, then the two L matmuls
                            # back-to-back, then the PVs.
                            thunks.append(lambda jt=jt - 1, f=ex0: f(jt))
                            thunks.append(lambda jt=jt - 1, f=ex1: f(jt))
                            thunks.append(lambda jt=jt - 1, f=el0: f(jt))
                            thunks.append(lambda jt=jt - 1, f=el1: f(jt))
                            thunks.append(lambda jt=jt - 1, f=ep0: f(jt))
                            thunks.append(lambda jt=jt - 1, f=ep1: f(jt))
                    thunks.append(n0)
                    thunks.append(n1)
                return thunks

            def outproj_thunks(ib):
                """Fine-grained: per token sub-block tt, 4 (ob) thunks of a
                4-matmul accumulation chain + ACT evacuation, then a DMA."""
                thunks = []

                def one(tt):
                    box = {}

                    def ob_thunk(ob):
                        def run():
                            if ob == 0:
                                box["stage"] = stage_tile()
                            ps = pp.tile([128, 512], F32, tag="st", name="ps")
                            for fc in range(NHL):
                                nc.tensor.matmul(
                                    ps[:],
                                    ctx[:, fc, ts(tt, 128)],
                                    wo_sb[:, fc, ts(ob, 512)],
                                    start=(fc == 0),
                                    stop=(fc == NHL - 1),
                                )
                            nc.scalar.copy(box["stage"][:, ts(ob, 512)], ps[:])
                            if not skip_outdma:
                                nc.sync.dma_start(
                                    out[tt, :, ts(ob, 512)],
                                    box["stage"][:, ts(ob, 512)],
                                )
                        return run

                    return [ob_thunk(ob) for ob in range(4)]

                for tt in range(4 * ib, 4 * ib + 4):
                    thunks.extend(one(tt))
                return thunks

            def run_interleaved(big, small):
                """Emit `small` steps spread evenly between `big` steps."""
                n, m = len(big), len(small)
                j = 0
                for i, b in enumerate(big):
                    b()
                    take = (m * (i + 1)) // n - j
                    for _ in range(take):
                        small[j]()
                        j += 1
                while j < m:
                    small[j]()
                    j += 1

            # prologue (outside the hw loop): blocks 0,1 for iteration 0
            if do_qkv and not dma_only:
                nc.sync.dma_start(xs_bufs[0][:], xT[:, 0])
                nc.sync.dma_start(xs_bufs[1][:], xT[:, 1])

            loop_cm = tc.For_i(0, loop, 1) if loop > 1 else contextlib.nullcontext()
            with loop_cm:
                for _rep in range(reps):
                    if dma_only:
                        for tb in range(TB):
                            nc.sync.dma_start(xs_bufs[tb % 3][:], xT[:, tb])
                        stage0 = stage_tile()
                        nc.vector.memset(stage0[:], 0.0)
                        for tt in range(JT):
                            nc.sync.dma_start(out[tt], stage0[:])
                        continue

                    if not do_qkv:
                        nc.vector.memset(qT[:], 0.0)
                        nc.vector.memset(kT[:], 0.0)
                        nc.vector.memset(vv[:], 0.0)
                    if not do_attn:
                        nc.vector.memset(ctx[:], 0.0)

                    A = (lambda ib: attn_thunks(ib)) if do_attn else (
                        lambda ib: []
                    )
                    O = (lambda ib: outproj_thunks(ib)) if do_outproj else (
                        lambda ib: []
                    )

                    if do_qkv:
                        xsA, xsB, xsC = xs_bufs
                        # blocks 0,1 already resident (prologue / previous
                        # iteration tail).  Load block 2 now.
                        nc.sync.dma_start(xsC[:], xT[:, 2])
                        # round 0: qkv blocks 0,1 dense
                        for t in qkv_thunks(0, xsA):
                            t()
                        for t in qkv_thunks(1, xsB):
                            t()
                        # round 1: attn(0) x qkv(2)
                        run_interleaved(A(0), qkv_thunks(2, xsC))
                        # block 3 reuses C (waits for qkv(2) reads)
                        nc.sync.dma_start(xsC[:], xT[:, 3])
                        # round 2: attn(1) x out(0) + qkv(3); out(0)
                        # leads so its matmuls hide the block-3 DMA
                        run_interleaved(A(1), O(0) + qkv_thunks(3, xsC))
                        # refill blocks 0,1 for the next hw-loop iteration
                        # (xT is loop-invariant, so this is idempotent)
                        nc.sync.dma_start(xsA[:], xT[:, 0])
                        nc.sync.dma_start(xsB[:], xT[:, 1])
                        # round 3: attn(2) x out(1)
                        run_interleaved(A(2), O(1))
                        # round 4: attn(3) x out(2)
                        run_interleaved(A(3), O(2))
                        # round 5: out(3)
                        for t in O(3):
                            t()
                    else:
                        for ib in range(IB):
                            for t in A(ib):
                                t()
                            for t in O(ib):
                                t()

    if split_waits:
        _split_all_multi_waits(nc)
    return nc


def _rope_tables():
    inv_freq = 1.0 / (10000.0 ** (np.arange(0, HD, 2, dtype=np.float32) / HD))
    t = np.arange(S, dtype=np.float32)
    freqs = np.einsum("i,j->ij", t, inv_freq)          # [S, 64]
    emb = np.concatenate([freqs, freqs], axis=-1)      # [S, 128]
    cos = np.cos(emb).T.astype(np.float32)             # [128, S]
    sin = np.sin(emb).T.astype(np.float32)             # [128, S]
    sin_signed = sin.copy()
    sin_signed[:64] *= -1.0                            # rotate_half sign fold
    return np.ascontiguousarray(cos), np.ascontiguousarray(sin_signed)


def _mask_diag():
    jj = np.arange(128)[:, None]
    ii = np.arange(128)[None, :]
    return np.where(ii >= jj, 0.0, NEG).astype(np.float32)


def _chunk_pmajor(a):
    """[R, C] with R = n*128 -> [128, n, C] with out[p, n, c] = a[n*128+p, c]."""
    n = a.shape[0] // 128
    return np.ascontiguousarray(a.reshape(n, 128, -1).transpose(1, 0, 2))


def make_in_maps(x, w_qkv, w_out):
    cos, sin_signed = _rope_tables()
    mask = _mask_diag()
    in_maps = []
    xT_by_b = []
    for b in range(B):
        # xT[p, tb, kc, t'] = x[b, tb*512+t', kc*128+p]
        xt = _chunk_pmajor(x[b].T.astype(np.float32))          # [128, KC, S]
        xt = xt.reshape(128, KC, TB, 512).transpose(0, 2, 1, 3)
        xT_by_b.append(np.ascontiguousarray(xt).astype(BF16))
    for c in range(8):
        b, hg = c // 4, c % 4
        rows = slice(hg * FQ, (hg + 1) * FQ)
        wq = _chunk_pmajor(w_qkv[0 * D:][rows].T).astype(BF16)   # [128, KC, FQ]
        wk = _chunk_pmajor(w_qkv[1 * D:][rows].T).astype(BF16)
        wv = _chunk_pmajor(w_qkv[2 * D:][rows].T).astype(BF16)
        wo = _chunk_pmajor(w_out[:, hg * FQ:(hg + 1) * FQ].T).astype(BF16)
        in_maps.append(
            {
                "xT": xT_by_b[b],
                "wqT": wq,
                "wkT": wk,
                "wvT": wv,
                "woT": wo,
                "cosT": cos.astype(BF16),
                "sinT": sin_signed.astype(BF16),
                "maskd": mask,
            }
        )
    return in_maps


_nc_cache = {}


def kernel(x, w_qkv, w_out):
    x = np.asarray(x)
    w_qkv = np.asarray(w_qkv)
    w_out = np.asarray(w_out)
    reps = int(os.environ.get("KERNEL_REPS", "1"))
    if reps not in _nc_cache:
        _nc_cache[reps] = build_nc(reps)
    nc = _nc_cache[reps]
    in_maps = make_in_maps(x, w_qkv, w_out)
    res = run_bass_kernel_spmd(nc, in_maps, list(range(8)), trace=False)
    out = np.zeros((B, S, D), dtype=np.float32)
    for c in range(8):
        out[c // 4] += res.results[c]["out"].reshape(S, D).astype(np.float32)
    return out


# revision 16
# speedup vs baseline: 1.0516x; 1.0516x over previous
"""Causal self-attention (B=2, S=2048, D=2048, 16 heads, RoPE) on 8 trn2 cores.

Sharding: tensor-parallel over heads x data-parallel over batch.
Core c handles batch b = c // 4 and head-group hg = c % 4 (heads 4*hg..4*hg+3).
qkv_proj is column-sharded by head, out_proj row-sharded by head; the
AllReduce of the out_proj partials is done on the host (4 partials per batch).

Per-core device program (all matmuls bf16 with fp32 PSUM accumulation).
The PE streams bf16 at ~0.52 ns/col sustained (P0 power state), with
LDWEIGHTS fully hidden, so runtime ~= total matmul output columns.  v2
therefore (a) trims the causal diagonal: S / exp / L / PV on query block
ib only compute columns [off:512] for diagonal-crossing key tiles
(off = 128*jt - 512*ib), saving ~37k of 770k columns; and (b) emits one
globally interleaved PE stream so exp (ACT) and rope (DVE) latency hides
behind independent matmuls:

  round 0: qkv token blocks 0,1 (dense)
  round 1: attn(ib0) interleaved with qkv block 2
  round 2: attn(ib1) interleaved with qkv block 3 + outproj(ib0)
  round 3: attn(ib2) interleaved with outproj(ib1)
  round 4: attn(ib3) interleaved with outproj(ib2)
  round 5: outproj(ib3)

attn runs heads in interleaved pairs (st x4 + o x2 + l x2 = 8 PSUM banks);
qkv/outproj sub-thunks of ~4 matmuls give ~1us interleave granularity.
"""

import contextlib
import math
import os

import numpy as np
import ml_dtypes

import bass_rust
import concourse.bass as bass
import concourse.mybir as mybir
import concourse.tile as tile
from concourse.bass import ts
from concourse.bass_utils import run_bass_kernel_spmd

BF16 = ml_dtypes.bfloat16
F32 = mybir.dt.float32
BF = mybir.dt.bfloat16

B = 2
S = 2048
D = 2048
HD = 128                    # head dim
NH = 16                     # total heads
NHL = 4                     # heads per core
FQ = NHL * HD               # 512 per-core q/k/v features
KC = D // 128               # 16 contraction chunks
TB = 4                      # token blocks of 512 (qkv phase)
IB = 4                      # query blocks of 512 (attention phase)
JT = S // 128               # 16 key tiles of 128
SCALE = 1.0 / math.sqrt(HD)
NEG = -30000.0              # additive mask; exp(NEG * SCALE) == 0 in fp32

MAX_WAITS = 1               # this walrus build allows 1 sync-wait per inst

_wait_ctr = [0]


def _split_all_multi_waits(nc):
    """This walrus build rejects instructions with >1 semaphore wait
    ("Too many sync wait commands").  Move extra waits onto NoOps inserted
    right before the instruction on the same engine (sequencers execute in
    order, so blocking one instruction earlier is equivalent)."""
    n_split = 0
    for f in nc.m.functions:
        for blk in f.blocks:
            out = []
            for inst in blk.instructions:
                si = inst.sync_info
                if si is not None and len(si.on_wait) > MAX_WAITS:
                    waits = list(si.on_wait)
                    for w in waits[:-MAX_WAITS]:
                        _wait_ctr[0] += 1
                        nop = mybir.InstNoOp(
                            name=f"I-waitsplit-{_wait_ctr[0]}", ins=[], outs=[]
                        )
                        nop.engine = inst.engine
                        nop.sync_info = bass_rust.SyncInfo(on_wait=[w], on_update=[])
                        out.append(nop)
                    inst.sync_info = bass_rust.SyncInfo(
                        on_wait=waits[-MAX_WAITS:], on_update=list(si.on_update)
                    )
                    n_split += 1
                out.append(inst)
            blk.instructions = out
    return n_split


def build_nc(
    reps: int = 1,
    split_waits: bool = True,
    loop: int = 1,
    do_qkv: bool = True,
    do_attn: bool = True,
    do_outproj: bool = True,
    dma_only: bool = False,
    skip_outdma: bool = False,
    split_outdma: bool = False,
):
    nc = bass.Bass()
    xT = nc.declare_dram_parameter("xT", [128, TB, KC, 512], BF, isOutput=False)
    wqT = nc.declare_dram_parameter("wqT", [128, KC, FQ], BF, isOutput=False)
    wkT = nc.declare_dram_parameter("wkT", [128, KC, FQ], BF, isOutput=False)
    wvT = nc.declare_dram_parameter("wvT", [128, KC, FQ], BF, isOutput=False)
    woT = nc.declare_dram_parameter("woT", [128, NHL, D], BF, isOutput=False)
    cosT = nc.declare_dram_parameter("cosT", [128, S], BF, isOutput=False)
    sinT = nc.declare_dram_parameter("sinT", [128, S], BF, isOutput=False)
    maskd = nc.declare_dram_parameter("maskd", [128, 128], F32, isOutput=False)
    out = nc.declare_dram_parameter("out", [JT, 128, D], BF, isOutput=True)

    mult = mybir.AluOpType.mult
    add = mybir.AluOpType.add
    EXP = mybir.ActivationFunctionType.Exp

    with tile.TileContext(nc) as tc:
        with (
            tc.tile_pool(name="persist", bufs=1) as persist,
            tc.tile_pool(name="stream", bufs=1) as stream,
            tc.tile_pool(name="pp", bufs=4, space="PSUM") as pp,
        ):
            qT = persist.tile([128, NHL, S], BF, tag="qT")
            kT = persist.tile([128, NHL, S], BF, tag="kT")
            vv = persist.tile([128, JT, FQ], BF, tag="vv")   # [t-part, tt, d]
            ctx = persist.tile([128, NHL, S], BF, tag="ctx")
            cos_sb = persist.tile([128, S], BF, tag="cos")
            sin_sb = persist.tile([128, S], BF, tag="sin")
            # three persistent x-block buffers: A=block0, B=block1,
            # C=blocks 2 then 3 (reloaded mid-iteration)
            xs_bufs = [
                persist.tile(
                    [128, KC, 512], BF, tag=f"xs{i}", name=f"xs{i}"
                )
                for i in range(3)
            ]
            mask_sb = persist.tile([128, 128], F32, tag="mask")
            ones_sb = persist.tile([128, 128], BF, tag="ones")
            wq_sb = persist.tile([128, KC, FQ], BF, tag="wq")
            wk_sb = persist.tile([128, KC, FQ], BF, tag="wk")
            wv_sb = persist.tile([128, KC, FQ], BF, tag="wv")
            wo_sb = persist.tile([128, NHL, D], BF, tag="wo")

            nc.sync.dma_start(cos_sb[:], cosT[:])
            nc.sync.dma_start(sin_sb[:], sinT[:])
            nc.sync.dma_start(mask_sb[:], maskd[:])
            nc.vector.memset(ones_sb[:], 1.0)
            nc.sync.dma_start(wq_sb[:], wqT[:])
            nc.sync.dma_start(wk_sb[:], wkT[:])
            nc.sync.dma_start(wv_sb[:], wvT[:])
            nc.sync.dma_start(wo_sb[:], woT[:])

            def stage_tile():
                return stream.tile(
                    [128, D], BF, tag="stage", bufs=2, name="stage"
                )

            def qkv_thunks(tb, xs):
                """Fine-grained thunks for token block tb: 12 chains
                (4 v + 4 q + 4 k), each split into 4 sub-thunks of 4
                accumulating matmuls plus an evacuation thunk."""
                tbs = ts(tb, 512)
                thunks = []

                def qk_chain(w_sb, dstT, f):
                    box = {}

                    def mm4(g):
                        def run():
                            if g == 0:
                                box["ps"] = pp.tile(
                                    [128, 512], F32, tag="st", name="ps"
                                )
                            ps = box["ps"]
                            for kc in range(4 * g, 4 * g + 4):
                                nc.tensor.matmul(
                                    ps[:],
                                    w_sb[:, kc, ts(f, 128)],
                                    xs[:, kc, :],
                                    start=(kc == 0),
                                    stop=(kc == KC - 1),
                                )
                        return run

                    def evac():
                        # rope: dst = ps*cos + swap(ps)*sin_signed
                        ps = box["ps"]
                        t1 = stream.tile([128, 512], BF, tag="t1", bufs=2)
                        nc.vector.tensor_tensor(
                            t1[:], ps[:], cos_sb[:, tbs], mult
                        )
                        t2 = stream.tile([128, 512], BF, tag="t2", bufs=2)
                        nc.vector.tensor_tensor(
                            t2[0:64, :], ps[64:128, :], sin_sb[0:64, tbs], mult
                        )
                        nc.vector.tensor_tensor(
                            t2[64:128, :], ps[0:64, :], sin_sb[64:128, tbs],
                            mult,
                        )
                        nc.vector.tensor_tensor(
                            dstT[:, f, tbs], t1[:], t2[:], add
                        )

                    return [mm4(g) for g in range(4)] + [evac]

                def v_chain(s4):
                    box = {}

                    def mm4(g):
                        def run():
                            if g == 0:
                                box["ps"] = pp.tile(
                                    [128, 512], F32, tag="st", name="ps"
                                )
                            ps = box["ps"]
                            for kc in range(4 * g, 4 * g + 4):
                                nc.tensor.matmul(
                                    ps[:],
                                    xs[:, kc, ts(s4, 128)],
                                    wv_sb[:, kc, :],
                                    start=(kc == 0),
                                    stop=(kc == KC - 1),
                                )
                        return run

                    def evac():
                        nc.scalar.copy(vv[:, tb * 4 + s4, :], box["ps"][:])

                    return [mm4(g) for g in range(4)] + [evac]

                # k/q chains for low heads first so attention on this
                # block can start as early as possible; v interleaved
                order = [
                    ("k", 0), ("q", 0), ("k", 1), ("q", 1),
                    ("v", 0), ("v", 1),
                    ("k", 2), ("q", 2), ("k", 3), ("q", 3),
                    ("v", 2), ("v", 3),
                ]
                for kind, idx in order:
                    if kind == "v":
                        thunks.extend(v_chain(idx))
                    elif kind == "q":
                        thunks.extend(qk_chain(wq_sb, qT, idx))
                    else:
                        thunks.extend(qk_chain(wk_sb, kT, idx))
                return thunks

            def attn_thunks(ib):
                """Step-thunks: flash attention for query block ib.  Heads
                run in interleaved PAIRS so each head's exp latency hides
                behind the other head's S/L/PV matmuls.  Diagonal-crossing
                key tiles only compute columns [off:512]."""
                njt = 4 * ib + 4
                thunks = []

                def make_head(h):
                    sts = [None] * njt
                    pts = [None] * njt
                    acc = {}

                    def off_of(jt):
                        o = (jt - 4 * ib) * 128
                        return o if o > 0 else 0

                    def start_head():
                        acc["o"] = pp.tile(
                            [128, 512], F32, tag="o", bufs=2, name="o"
                        )
                        acc["l"] = pp.tile(
                            [128, 512], F32, tag="l", bufs=2, name="l"
                        )

                    def emit_s(jt):
                        off = off_of(jt)
                        st = pp.tile([128, 512], F32, tag="st", name="st")
                        nc.tensor.matmul(
                            st[:, off:512],
                            kT[:, h, ts(jt, 128)],
                            qT[:, h, ib * 512 + off : ib * 512 + 512],
                            start=True,
                            stop=True,
                        )
                        sts[jt] = st

                    def emit_exp(jt):
                        st = sts[jt]
                        off = off_of(jt)
                        pt = stream.tile([128, 512], BF, tag="pt", bufs=6)
                        if jt >= 4 * ib:
                            nc.vector.tensor_tensor(
                                st[:, off : off + 128],
                                st[:, off : off + 128],
                                mask_sb[:],
                                add,
                            )
                        nc.scalar.activation(
                            pt[:, off:512], st[:, off:512], EXP, scale=SCALE
                        )
                        pts[jt] = pt

                    def emit_l(jt):
                        off = off_of(jt)
                        nc.tensor.matmul(
                            acc["l"][:, off:512], ones_sb[:],
                            pts[jt][:, off:512],
                            start=(jt == 0), stop=(jt == njt - 1),
                        )

                    def emit_pv(jt):
                        off = off_of(jt)
                        nc.tensor.matmul(
                            acc["o"][:, off:512], vv[:, jt, ts(h, 128)],
                            pts[jt][:, off:512],
                            start=(jt == 0), stop=(jt == njt - 1),
                        )

                    def norm_head():
                        linv = stream.tile([128, 512], F32, tag="linv", bufs=2)
                        nc.vector.reciprocal(linv[:], acc["l"][:])
                        nc.vector.tensor_tensor(
                            ctx[:, h, ts(ib, 512)], acc["o"][:], linv[:], mult
                        )

                    return start_head, emit_s, emit_exp, emit_l, emit_pv, \
                        norm_head

                for hp in (0, 2):
                    s0, es0, ex0, el0, ep0, n0 = make_head(hp)
                    s1, es1, ex1, el1, ep1, n1 = make_head(hp + 1)
                    thunks.append(s0)
                    thunks.append(s1)
                    for jt in range(njt + 2):
                        if jt < njt:
                            thunks.append(lambda jt=jt, f=es0: f(jt))
                            thunks.append(lambda jt=jt, f=es1: f(jt))
                        if 1 <= jt <= njt:
                            # exp one tile behind S
                            thunks.append(lambda jt=jt - 1, f=ex0: f(jt))
                            thunks.append(lambda jt=jt - 1, f=ex1: f(jt))
                        if jt >= 2:
                            # L/PV two tiles behind S: a full round of slack
                            # for the ACT exp and its semaphore hops
                            thunks.append(lambda jt=jt - 2, f=el0: f(jt))
                            thunks.append(lambda jt=jt - 2, f=el1: f(jt))
                            thunks.append(lambda jt=jt - 2, f=ep0: f(jt))
                            thunks.append(lambda jt=jt - 2, f=ep1: f(jt))
                    thunks.append(n0)
                    thunks.append(n1)
                return thunks

            def outproj_thunks(ib):
                """Fine-grained: per token sub-block tt, 4 (ob) thunks of a
                4-matmul accumulation chain + ACT evacuation, then a DMA."""
                thunks = []

                def one(tt):
                    box = {}

                    def ob_thunk(ob):
                        def run():
                            if ob == 0:
                                box["stage"] = stage_tile()
                            ps = pp.tile([128, 512], F32, tag="st", name="ps")
                            for fc in range(NHL):
                                nc.tensor.matmul(
                                    ps[:],
                                    ctx[:, fc, ts(tt, 128)],
                                    wo_sb[:, fc, ts(ob, 512)],
                                    start=(fc == 0),
                                    stop=(fc == NHL - 1),
                                )
                            nc.scalar.copy(box["stage"][:, ts(ob, 512)], ps[:])
                            if split_outdma and not skip_outdma:
                                nc.sync.dma_start(
                                    out[tt, :, ts(ob, 512)],
                                    box["stage"][:, ts(ob, 512)],
                                )
                        return run

                    def dma():
                        if not skip_outdma:
                            nc.sync.dma_start(out[tt], box["stage"][:])

                    thl = [ob_thunk(ob) for ob in range(4)]
                    if not split_outdma:
                        thl.append(dma)
                    return thl

                for tt in range(4 * ib, 4 * ib + 4):
                    thunks.extend(one(tt))
                return thunks

            def run_interleaved(big, small):
                """Emit `small` steps spread evenly between `big` steps."""
                n, m = len(big), len(small)
                j = 0
                for i, b in enumerate(big):
                    b()
                    take = (m * (i + 1)) // n - j
                    for _ in range(take):
                        small[j]()
                        j += 1
                while j < m:
                    small[j]()
                    j += 1

            # prologue (outside the hw loop): blocks 0,1 for iteration 0
            if do_qkv and not dma_only:
                nc.sync.dma_start(xs_bufs[0][:], xT[:, 0])
                nc.sync.dma_start(xs_bufs[1][:], xT[:, 1])

            loop_cm = tc.For_i(0, loop, 1) if loop > 1 else contextlib.nullcontext()
            with loop_cm:
                for _rep in range(reps):
                    if dma_only:
                        for tb in range(TB):
                            nc.sync.dma_start(xs_bufs[tb % 3][:], xT[:, tb])
                        stage0 = stage_tile()
                        nc.vector.memset(stage0[:], 0.0)
                        for tt in range(JT):
                            nc.sync.dma_start(out[tt], stage0[:])
                        continue

                    if not do_qkv:
                        nc.vector.memset(qT[:], 0.0)
                        nc.vector.memset(kT[:], 0.0)
                        nc.vector.memset(vv[:], 0.0)
                    if not do_attn:
                        nc.vector.memset(ctx[:], 0.0)

                    A = (lambda ib: attn_thunks(ib)) if do_attn else (
                        lambda ib: []
                    )
                    O = (lambda ib: outproj_thunks(ib)) if do_outproj else (
                        lambda ib: []
                    )

                    if do_qkv:
                        xsA, xsB, xsC = xs_bufs
                        # blocks 0,1 already resident (prologue / previous
                        # iteration tail).  Load block 2 now.
                        nc.sync.dma_start(xsC[:], xT[:, 2])
                        # round 0: qkv blocks 0,1 dense
                        for t in qkv_thunks(0, xsA):
                            t()
                        for t in qkv_thunks(1, xsB):
                            t()
                        # round 1: attn(0) x qkv(2)
                        run_interleaved(A(0), qkv_thunks(2, xsC))
                        # block 3 reuses C (waits for qkv(2) reads)
                        nc.sync.dma_start(xsC[:], xT[:, 3])
                        # round 2: attn(1) x out(0) + qkv(3); out(0)
                        # leads so its matmuls hide the block-3 DMA
                        run_interleaved(A(1), O(0) + qkv_thunks(3, xsC))
                        # refill blocks 0,1 for the next hw-loop iteration
                        # (xT is loop-invariant, so this is idempotent)
                        nc.sync.dma_start(xsA[:], xT[:, 0])
                        nc.sync.dma_start(xsB[:], xT[:, 1])
                        # round 3: attn(2) x out(1)
                        run_interleaved(A(2), O(1))
                        # round 4: attn(3) x out(2)
                        run_interleaved(A(3), O(2))
                        # round 5: out(3)
                        for t in O(3):
                            t()
                    else:
                        for ib in range(IB):
                            for t in A(ib):
                                t()
                            for t in O(ib):
                                t()

    if split_waits:
        _split_all_multi_waits(nc)
    return nc


def _rope_tables():
    inv_freq = 1.0 / (10000.0 ** (np.arange(0, HD, 2, dtype=np.float32) / HD))
    t = np.arange(S, dtype=np.float32)
    freqs = np.einsum("i,j->ij", t, inv_freq)          # [S, 64]
    emb = np.concatenate([freqs, freqs], axis=-1)      # [S, 128]
    cos = np.cos(emb).T.astype(np.float32)             # [128, S]
    sin = np.sin(emb).T.astype(np.float32)             # [128, S]
    sin_signed = sin.copy()
    sin_signed[:64] *= -1.0                            # rotate_half sign fold
    return np.ascontiguousarray(cos), np.ascontiguousarray(sin_signed)


def _mask_diag():
    jj = np.arange(128)[:, None]
    ii = np.arange(128)[None, :]
    return np.where(ii >= jj, 0.0, NEG).astype(np.float32)


def _chunk_pmajor(a):
    """[R, C] with R = n*128 -> [128, n, C] with out[p, n, c] = a[n*128+p, c]."""
    n = a.shape[0] // 128
    return np.ascontiguousarray(a.reshape(n, 128, -1).transpose(1, 0, 2))


def make_in_maps(x, w_qkv, w_out):
    cos, sin_signed = _rope_tables()
    mask = _mask_diag()
    in_maps = []
    xT_by_b = []
    for b in range(B):
        # xT[p, tb, kc, t'] = x[b, tb*512+t', kc*128+p]
        xt = _chunk_pmajor(x[b].T.astype(np.float32))          # [128, KC, S]
        xt = xt.reshape(128, KC, TB, 512).transpose(0, 2, 1, 3)
        xT_by_b.append(np.ascontiguousarray(xt).astype(BF16))
    for c in range(8):
        b, hg = c // 4, c % 4
        rows = slice(hg * FQ, (hg + 1) * FQ)
        wq = _chunk_pmajor(w_qkv[0 * D:][rows].T).astype(BF16)   # [128, KC, FQ]
        wk = _chunk_pmajor(w_qkv[1 * D:][rows].T).astype(BF16)
        wv = _chunk_pmajor(w_qkv[2 * D:][rows].T).astype(BF16)
        wo = _chunk_pmajor(w_out[:, hg * FQ:(hg + 1) * FQ].T).astype(BF16)
        in_maps.append(
            {
                "xT": xT_by_b[b],
                "wqT": wq,
                "wkT": wk,
                "wvT": wv,
                "woT": wo,
                "cosT": cos.astype(BF16),
                "sinT": sin_signed.astype(BF16),
                "maskd": mask,
            }
        )
    return in_maps


_nc_cache = {}


def kernel(x, w_qkv, w_out):
    x = np.asarray(x)
    w_qkv = np.asarray(w_qkv)
    w_out = np.asarray(w_out)
    reps = int(os.environ.get("KERNEL_REPS", "1"))
    if reps not in _nc_cache:
        _nc_cache[reps] = build_nc(reps)
    nc = _nc_cache[reps]
    in_maps = make_in_maps(x, w_qkv, w_out)
    res = run_bass_kernel_spmd(nc, in_maps, list(range(8)), trace=False)
    out = np.zeros((B, S, D), dtype=np.float32)
    for c in range(8):
        out[c // 4] += res.results[c]["out"].reshape(S, D).astype(np.float32)
    return out


# revision 20
# speedup vs baseline: 1.1426x; 1.0865x over previous
"""Causal self-attention (B=2, S=2048, D=2048, 16 heads, RoPE) on 8 trn2 cores.

Sharding: tensor-parallel over heads x data-parallel over batch.
Core c handles batch b = c // 4 and head-group hg = c % 4 (heads 4*hg..4*hg+3).
qkv_proj is column-sharded by head, out_proj row-sharded by head; the
AllReduce of the out_proj partials is done on the host (4 partials per batch).

Per-core device program (all matmuls bf16 with fp32 PSUM accumulation).
HW-measured: the PE streams bf16 at ~0.52 ns/col sustained (power-state
downclock ~2.0 GHz; fp8 would stream at 2.4 but fails the 2e-2 gate by
2-5x at every site), with LDWEIGHTS fully hidden and weight reuse
irrelevant, so runtime ~= total matmul output columns (~733k).  Design:

  (a) causal diagonal trim: S / exp / L / PV for diagonal-crossing key
      tiles compute only columns [off:512], off = 128*jt - 512*ib
      (saves 37k of 770k columns; L = softmax denominator via
      ones-matmul, the cheapest partition reduction on this chip);
  (b) one globally interleaved PE stream; independent qkv/outproj
      matmul sub-thunks (~4 MMs) fill attention dependency bubbles:
        round 0: qkv blocks 0,1 (dense)   round 3: attn(2) x out(1)
        round 1: attn(0) x qkv(2)         round 4: attn(3) x out(2)
        round 2: attn(1) x out(0)+qkv(3)  round 5: out(3)
  (c) attention heads in interleaved pairs (st x4 + o x2 + l x2 = 8
      PSUM banks), exp one tile behind S and L/PV two tiles behind, so
      ACT latency and both semaphore hops are off the PE critical path;
  (d) x blocks 0,1 live in persistent buffers refilled mid-iteration
      for the NEXT hw-loop iteration (xT is loop-invariant), block 2/3
      share the third buffer; kills the top-of-iteration DMA stall.

Steady state: sim ~313 us/iter (PE busy 307 us, gaps ~3 us); HW ~405 us
cold, ~430-455 us hot (the chip heats over ~1 s of sustained work and
the clock drops; medians depend on thermal history).  Baseline was
494 us; measured A/B delta vs baseline is -55 us (0.888x).
"""

import contextlib
import math
import os

import numpy as np
import ml_dtypes

import bass_rust
import concourse.bass as bass
import concourse.mybir as mybir
import concourse.tile as tile
from concourse.bass import ts
from concourse.bass_utils import run_bass_kernel_spmd

BF16 = ml_dtypes.bfloat16
F32 = mybir.dt.float32
BF = mybir.dt.bfloat16

B = 2
S = 2048
D = 2048
HD = 128                    # head dim
NH = 16                     # total heads
NHL = 4                     # heads per core
FQ = NHL * HD               # 512 per-core q/k/v features
KC = D // 128               # 16 contraction chunks
TB = 4                      # token blocks of 512 (qkv phase)
IB = 4                      # query blocks of 512 (attention phase)
JT = S // 128               # 16 key tiles of 128
SCALE = 1.0 / math.sqrt(HD)
NEG = -30000.0              # additive mask; exp(NEG * SCALE) == 0 in fp32

MAX_WAITS = 1               # this walrus build allows 1 sync-wait per inst

_wait_ctr = [0]


def _split_all_multi_waits(nc):
    """This walrus build rejects instructions with >1 semaphore wait
    ("Too many sync wait commands").  Move extra waits onto NoOps inserted
    right before the instruction on the same engine (sequencers execute in
    order, so blocking one instruction earlier is equivalent)."""
    n_split = 0
    for f in nc.m.functions:
        for blk in f.blocks:
            out = []
            for inst in blk.instructions:
                si = inst.sync_info
                if si is not None and len(si.on_wait) > MAX_WAITS:
                    waits = list(si.on_wait)
                    for w in waits[:-MAX_WAITS]:
                        _wait_ctr[0] += 1
                        nop = mybir.InstNoOp(
                            name=f"I-waitsplit-{_wait_ctr[0]}", ins=[], outs=[]
                        )
                        nop.engine = inst.engine
                        nop.sync_info = bass_rust.SyncInfo(on_wait=[w], on_update=[])
                        out.append(nop)
                    inst.sync_info = bass_rust.SyncInfo(
                        on_wait=waits[-MAX_WAITS:], on_update=list(si.on_update)
                    )
                    n_split += 1
                out.append(inst)
            blk.instructions = out
    return n_split


def build_nc(
    reps: int = 1,
    split_waits: bool = True,
    loop: int = 1,
    do_qkv: bool = True,
    do_attn: bool = True,
    do_outproj: bool = True,
    dma_only: bool = False,
    skip_outdma: bool = False,
    split_outdma: bool = False,
    out_evac_dve: bool = False,
):
    nc = bass.Bass()
    xT = nc.declare_dram_parameter("xT", [128, TB, KC, 512], BF, isOutput=False)
    wqT = nc.declare_dram_parameter("wqT", [128, KC, FQ], BF, isOutput=False)
    wkT = nc.declare_dram_parameter("wkT", [128, KC, FQ], BF, isOutput=False)
    wvT = nc.declare_dram_parameter("wvT", [128, KC, FQ], BF, isOutput=False)
    woT = nc.declare_dram_parameter("woT", [128, NHL, D], BF, isOutput=False)
    cosT = nc.declare_dram_parameter("cosT", [128, S], BF, isOutput=False)
    sinT = nc.declare_dram_parameter("sinT", [128, S], BF, isOutput=False)
    maskd = nc.declare_dram_parameter("maskd", [128, 128], F32, isOutput=False)
    out = nc.declare_dram_parameter("out", [JT, 128, D], BF, isOutput=True)

    mult = mybir.AluOpType.mult
    add = mybir.AluOpType.add
    EXP = mybir.ActivationFunctionType.Exp

    with tile.TileContext(nc) as tc:
        with (
            tc.tile_pool(name="persist", bufs=1) as persist,
            tc.tile_pool(name="stream", bufs=1) as stream,
            tc.tile_pool(name="pp", bufs=4, space="PSUM") as pp,
        ):
            qT = persist.tile([128, NHL, S], BF, tag="qT")
            kT = persist.tile([128, NHL, S], BF, tag="kT")
            vv = persist.tile([128, JT, FQ], BF, tag="vv")   # [t-part, tt, d]
            ctx = persist.tile([128, NHL, S], BF, tag="ctx")
            cos_sb = persist.tile([128, S], BF, tag="cos")
            sin_sb = persist.tile([128, S], BF, tag="sin")
            # three persistent x-block buffers: A=block0, B=block1,
            # C=blocks 2 then 3 (reloaded mid-iteration)
            xs_bufs = [
                persist.tile(
                    [128, KC, 512], BF, tag=f"xs{i}", name=f"xs{i}"
                )
                for i in range(3)
            ]
            mask_sb = persist.tile([128, 128], F32, tag="mask")
            ones_sb = persist.tile([128, 128], BF, tag="ones")
            wq_sb = persist.tile([128, KC, FQ], BF, tag="wq")
            wk_sb = persist.tile([128, KC, FQ], BF, tag="wk")
            wv_sb = persist.tile([128, KC, FQ], BF, tag="wv")
            wo_sb = persist.tile([128, NHL, D], BF, tag="wo")

            nc.sync.dma_start(cos_sb[:], cosT[:])
            nc.sync.dma_start(sin_sb[:], sinT[:])
            nc.sync.dma_start(mask_sb[:], maskd[:])
            nc.vector.memset(ones_sb[:], 1.0)
            nc.sync.dma_start(wq_sb[:], wqT[:])
            nc.sync.dma_start(wk_sb[:], wkT[:])
            nc.sync.dma_start(wv_sb[:], wvT[:])
            nc.sync.dma_start(wo_sb[:], woT[:])

            def stage_tile():
                return stream.tile(
                    [128, D], BF, tag="stage", bufs=2, name="stage"
                )

            def qkv_thunks(tb, xs):
                """Fine-grained thunks for token block tb: 12 chains
                (4 v + 4 q + 4 k), each split into 4 sub-thunks of 4
                accumulating matmuls plus an evacuation thunk."""
                tbs = ts(tb, 512)
                thunks = []

                def qk_chain(w_sb, dstT, f):
                    box = {}

                    def mm4(g):
                        def run():
                            if g == 0:
                                box["ps"] = pp.tile(
                                    [128, 512], F32, tag="st", name="ps"
                                )
                            ps = box["ps"]
                            for kc in range(4 * g, 4 * g + 4):
                                nc.tensor.matmul(
                                    ps[:],
                                    w_sb[:, kc, ts(f, 128)],
                                    xs[:, kc, :],
                                    start=(kc == 0),
                                    stop=(kc == KC - 1),
                                )
                        return run

                    def evac():
                        # rope: dst = ps*cos + swap(ps)*sin_signed
                        ps = box["ps"]
                        t1 = stream.tile([128, 512], BF, tag="t1", bufs=2)
                        nc.vector.tensor_tensor(
                            t1[:], ps[:], cos_sb[:, tbs], mult
                        )
                        t2 = stream.tile([128, 512], BF, tag="t2", bufs=2)
                        nc.vector.tensor_tensor(
                            t2[0:64, :], ps[64:128, :], sin_sb[0:64, tbs], mult
                        )
                        nc.vector.tensor_tensor(
                            t2[64:128, :], ps[0:64, :], sin_sb[64:128, tbs],
                            mult,
                        )
                        nc.vector.tensor_tensor(
                            dstT[:, f, tbs], t1[:], t2[:], add
                        )

                    return [mm4(g) for g in range(4)] + [evac]

                def v_chain(s4):
                    box = {}

                    def mm4(g):
                        def run():
                            if g == 0:
                                box["ps"] = pp.tile(
                                    [128, 512], F32, tag="st", name="ps"
                                )
                            ps = box["ps"]
                            for kc in range(4 * g, 4 * g + 4):
                                nc.tensor.matmul(
                                    ps[:],
                                    xs[:, kc, ts(s4, 128)],
                                    wv_sb[:, kc, :],
                                    start=(kc == 0),
                                    stop=(kc == KC - 1),
                                )
                        return run

                    def evac():
                        nc.scalar.copy(vv[:, tb * 4 + s4, :], box["ps"][:])

                    return [mm4(g) for g in range(4)] + [evac]

                # k/q chains for low heads first so attention on this
                # block can start as early as possible; v interleaved
                order = [
                    ("k", 0), ("q", 0), ("k", 1), ("q", 1),
                    ("v", 0), ("v", 1),
                    ("k", 2), ("q", 2), ("k", 3), ("q", 3),
                    ("v", 2), ("v", 3),
                ]
                for kind, idx in order:
                    if kind == "v":
                        thunks.extend(v_chain(idx))
                    elif kind == "q":
                        thunks.extend(qk_chain(wq_sb, qT, idx))
                    else:
                        thunks.extend(qk_chain(wk_sb, kT, idx))
                return thunks

            def attn_thunks(ib):
                """Step-thunks: flash attention for query block ib.  Heads
                run in interleaved PAIRS so each head's exp latency hides
                behind the other head's S/L/PV matmuls.  Diagonal-crossing
                key tiles only compute columns [off:512]."""
                njt = 4 * ib + 4
                thunks = []

                def make_head(h):
                    sts = [None] * njt
                    pts = [None] * njt
                    acc = {}

                    def off_of(jt):
                        o = (jt - 4 * ib) * 128
                        return o if o > 0 else 0

                    def start_head():
                        acc["o"] = pp.tile(
                            [128, 512], F32, tag="o", bufs=2, name="o"
                        )
                        acc["l"] = pp.tile(
                            [128, 512], F32, tag="l", bufs=2, name="l"
                        )

                    def emit_s(jt):
                        off = off_of(jt)
                        st = pp.tile([128, 512], F32, tag="st", name="st")
                        nc.tensor.matmul(
                            st[:, off:512],
                            kT[:, h, ts(jt, 128)],
                            qT[:, h, ib * 512 + off : ib * 512 + 512],
                            start=True,
                            stop=True,
                        )
                        sts[jt] = st

                    def emit_exp(jt):
                        st = sts[jt]
                        off = off_of(jt)
                        pt = stream.tile([128, 512], BF, tag="pt", bufs=6)
                        if jt >= 4 * ib:
                            nc.vector.tensor_tensor(
                                st[:, off : off + 128],
                                st[:, off : off + 128],
                                mask_sb[:],
                                add,
                            )
                        nc.scalar.activation(
                            pt[:, off:512], st[:, off:512], EXP, scale=SCALE
                        )
                        pts[jt] = pt

                    def emit_l(jt):
                        off = off_of(jt)
                        nc.tensor.matmul(
                            acc["l"][:, off:512], ones_sb[:],
                            pts[jt][:, off:512],
                            start=(jt == 0), stop=(jt == njt - 1),
                        )

                    def emit_pv(jt):
                        off = off_of(jt)
                        nc.tensor.matmul(
                            acc["o"][:, off:512], vv[:, jt, ts(h, 128)],
                            pts[jt][:, off:512],
                            start=(jt == 0), stop=(jt == njt - 1),
                        )

                    def norm_head():
                        linv = stream.tile([128, 512], F32, tag="linv", bufs=2)
                        nc.vector.reciprocal(linv[:], acc["l"][:])
                        nc.vector.tensor_tensor(
                            ctx[:, h, ts(ib, 512)], acc["o"][:], linv[:], mult
                        )

                    return start_head, emit_s, emit_exp, emit_l, emit_pv, \
                        norm_head

                for hp in (0, 2):
                    s0, es0, ex0, el0, ep0, n0 = make_head(hp)
                    s1, es1, ex1, el1, ep1, n1 = make_head(hp + 1)
                    thunks.append(s0)
                    thunks.append(s1)
                    for jt in range(njt + 2):
                        if jt < njt:
                            thunks.append(lambda jt=jt, f=es0: f(jt))
                            thunks.append(lambda jt=jt, f=es1: f(jt))
                        if 1 <= jt <= njt:
                            # exp one tile behind S
                            thunks.append(lambda jt=jt - 1, f=ex0: f(jt))
                            thunks.append(lambda jt=jt - 1, f=ex1: f(jt))
                        if jt >= 2:
                            # L/PV two tiles behind S: a full round of slack
                            # for the ACT exp and its semaphore hops
                            thunks.append(lambda jt=jt - 2, f=el0: f(jt))
                            thunks.append(lambda jt=jt - 2, f=el1: f(jt))
                            thunks.append(lambda jt=jt - 2, f=ep0: f(jt))
                            thunks.append(lambda jt=jt - 2, f=ep1: f(jt))
                    thunks.append(n0)
                    thunks.append(n1)
                return thunks

            def outproj_thunks(ib):
                """Fine-grained: per token sub-block tt, 4 (ob) thunks of a
                4-matmul accumulation chain + ACT evacuation, then a DMA."""
                thunks = []

                def one(tt):
                    box = {}

                    def ob_thunk(ob):
                        def run():
                            if ob == 0:
                                box["stage"] = stage_tile()
                            ps = pp.tile([128, 512], F32, tag="st", name="ps")
                            for fc in range(NHL):
                                nc.tensor.matmul(
                                    ps[:],
                                    ctx[:, fc, ts(tt, 128)],
                                    wo_sb[:, fc, ts(ob, 512)],
                                    start=(fc == 0),
                                    stop=(fc == NHL - 1),
                                )
                            if out_evac_dve:
                                # DVE evacuation: keeps the ACT FIFO free
                                # for exps in the attention rounds and
                                # releases the PSUM bank sooner
                                nc.vector.tensor_scalar_add(
                                    box["stage"][:, ts(ob, 512)], ps[:], 0.0
                                )
                            else:
                                nc.scalar.copy(
                                    box["stage"][:, ts(ob, 512)], ps[:]
                                )
                            if split_outdma and not skip_outdma:
                                nc.sync.dma_start(
                                    out[tt, :, ts(ob, 512)],
                                    box["stage"][:, ts(ob, 512)],
                                )
                        return run

                    def dma():
                        if not skip_outdma:
                            nc.sync.dma_start(out[tt], box["stage"][:])

                    thl = [ob_thunk(ob) for ob in range(4)]
                    if not split_outdma:
                        thl.append(dma)
                    return thl

                for tt in range(4 * ib, 4 * ib + 4):
                    thunks.extend(one(tt))
                return thunks

            def run_interleaved(big, small):
                """Emit `small` steps spread evenly between `big` steps."""
                n, m = len(big), len(small)
                j = 0
                for i, b in enumerate(big):
                    b()
                    take = (m * (i + 1)) // n - j
                    for _ in range(take):
                        small[j]()
                        j += 1
                while j < m:
                    small[j]()
                    j += 1

            # prologue (outside the hw loop): blocks 0,1 for iteration 0
            if do_qkv and not dma_only:
                nc.sync.dma_start(xs_bufs[0][:], xT[:, 0])
                nc.sync.dma_start(xs_bufs[1][:], xT[:, 1])

            loop_cm = tc.For_i(0, loop, 1) if loop > 1 else contextlib.nullcontext()
            with loop_cm:
                for _rep in range(reps):
                    if dma_only:
                        for tb in range(TB):
                            nc.sync.dma_start(xs_bufs[tb % 3][:], xT[:, tb])
                        stage0 = stage_tile()
                        nc.vector.memset(stage0[:], 0.0)
                        for tt in range(JT):
                            nc.sync.dma_start(out[tt], stage0[:])
                        continue

                    if not do_qkv:
                        nc.vector.memset(qT[:], 0.0)
                        nc.vector.memset(kT[:], 0.0)
                        nc.vector.memset(vv[:], 0.0)
                    if not do_attn:
                        nc.vector.memset(ctx[:], 0.0)

                    A = (lambda ib: attn_thunks(ib)) if do_attn else (
                        lambda ib: []
                    )
                    O = (lambda ib: outproj_thunks(ib)) if do_outproj else (
                        lambda ib: []
                    )

                    if do_qkv:
                        xsA, xsB, xsC = xs_bufs
                        # blocks 0,1 already resident (prologue / previous
                        # iteration tail).  Load block 2 now.
                        nc.sync.dma_start(xsC[:], xT[:, 2])
                        # round 0: qkv blocks 0,1 dense
                        for t in qkv_thunks(0, xsA):
                            t()
                        for t in qkv_thunks(1, xsB):
                            t()
                        # round 1: attn(0) x qkv(2)
                        run_interleaved(A(0), qkv_thunks(2, xsC))
                        # block 3 reuses C (waits for qkv(2) reads)
                        nc.sync.dma_start(xsC[:], xT[:, 3])
                        # round 2: attn(1) x out(0)[:2 chains] + qkv(3);
                        # out(0) leads so its matmuls hide the block-3 DMA
                        o0 = O(0)
                        run_interleaved(A(1), o0[:10] + qkv_thunks(3, xsC))
                        # refill blocks 0,1 for the next hw-loop iteration
                        # (xT is loop-invariant, so this is idempotent)
                        nc.sync.dma_start(xsA[:], xT[:, 0])
                        nc.sync.dma_start(xsB[:], xT[:, 1])
                        # round 3: attn(2) x out(1)
                        run_interleaved(A(2), O(1))
                        # round 4: attn(3) x out(0)[2:] + out(2)
                        run_interleaved(A(3), o0[10:] + O(2))
                        # round 5: out(3)
                        for t in O(3):
                            t()
                    else:
                        for ib in range(IB):
                            for t in A(ib):
                                t()
                            for t in O(ib):
                                t()

    if split_waits:
        _split_all_multi_waits(nc)
    return nc


def _rope_tables():
    inv_freq = 1.0 / (10000.0 ** (np.arange(0, HD, 2, dtype=np.float32) / HD))
    t = np.arange(S, dtype=np.float32)
    freqs = np.einsum("i,j->ij", t, inv_freq)          # [S, 64]
    emb = np.concatenate([freqs, freqs], axis=-1)      # [S, 128]
    cos = np.cos(emb).T.astype(np.float32)             # [128, S]
    sin = np.sin(emb).T.astype(np.float32)             # [128, S]
    sin_signed = sin.copy()
    sin_signed[:64] *= -1.0                            # rotate_half sign fold
    return np.ascontiguousarray(cos), np.ascontiguousarray(sin_signed)


def _mask_diag():
    jj = np.arange(128)[:, None]
    ii = np.arange(128)[None, :]
    return np.where(ii >= jj, 0.0, NEG).astype(np.float32)


def _chunk_pmajor(a):
    """[R, C] with R = n*128 -> [128, n, C] with out[p, n, c] = a[n*128+p, c]."""
    n = a.shape[0] // 128
    return np.ascontiguousarray(a.reshape(n, 128, -1).transpose(1, 0, 2))


def make_in_maps(x, w_qkv, w_out):
    cos, sin_signed = _rope_tables()
    mask = _mask_diag()
    in_maps = []
    xT_by_b = []
    for b in range(B):
        # xT[p, tb, kc, t'] = x[b, tb*512+t', kc*128+p]
        xt = _chunk_pmajor(x[b].T.astype(np.float32))          # [128, KC, S]
        xt = xt.reshape(128, KC, TB, 512).transpose(0, 2, 1, 3)
        xT_by_b.append(np.ascontiguousarray(xt).astype(BF16))
    for c in range(8):
        b, hg = c // 4, c % 4
        rows = slice(hg * FQ, (hg + 1) * FQ)
        wq = _chunk_pmajor(w_qkv[0 * D:][rows].T).astype(BF16)   # [128, KC, FQ]
        wk = _chunk_pmajor(w_qkv[1 * D:][rows].T).astype(BF16)
        wv = _chunk_pmajor(w_qkv[2 * D:][rows].T).astype(BF16)
        wo = _chunk_pmajor(w_out[:, hg * FQ:(hg + 1) * FQ].T).astype(BF16)
        in_maps.append(
            {
                "xT": xT_by_b[b],
                "wqT": wq,
                "wkT": wk,
                "wvT": wv,
                "woT": wo,
                "cosT": cos.astype(BF16),
                "sinT": sin_signed.astype(BF16),
                "maskd": mask,
            }
        )
    return in_maps


_nc_cache = {}


def kernel(x, w_qkv, w_out):
    x = np.asarray(x)
    w_qkv = np.asarray(w_qkv)
    w_out = np.asarray(w_out)
    reps = int(os.environ.get("KERNEL_REPS", "1"))
    if reps not in _nc_cache:
        _nc_cache[reps] = build_nc(reps)
    nc = _nc_cache[reps]
    in_maps = make_in_maps(x, w_qkv, w_out)
    res = run_bass_kernel_spmd(nc, in_maps, list(range(8)), trace=False)
    out = np.zeros((B, S, D), dtype=np.float32)
    for c in range(8):
        out[c // 4] += res.results[c]["out"].reshape(S, D).astype(np.float32)
    return out


# revision 21
# speedup vs baseline: 1.1617x; 1.0168x over previous
"""Causal self-attention (B=2, S=2048, D=2048, 16 heads, RoPE) on 8 trn2 cores.

Sharding: tensor-parallel over heads x data-parallel over batch.
Core c handles batch b = c // 4 and head-group hg = c % 4 (heads 4*hg..4*hg+3).
qkv_proj is column-sharded by head, out_proj row-sharded by head; the
AllReduce of the out_proj partials is done on the host (4 partials per batch).

Per-core device program (all matmuls bf16 with fp32 PSUM accumulation).
HW-measured: the PE streams bf16 at ~0.52 ns/col sustained (power-state
downclock ~2.0 GHz; fp8 would stream at 2.4 but fails the 2e-2 gate by
2-5x at every site), with LDWEIGHTS fully hidden and weight reuse
irrelevant, so runtime ~= total matmul output columns (~733k).  Design:

  (a) causal diagonal trim: S / exp / L / PV for diagonal-crossing key
      tiles compute only columns [off:512], off = 128*jt - 512*ib
      (saves 37k of 770k columns; L = softmax denominator via
      ones-matmul, the cheapest partition reduction on this chip);
  (b) one globally interleaved PE stream; independent qkv/outproj
      matmul sub-thunks (~4 MMs) fill attention dependency bubbles:
        round 0: qkv blocks 0,1 (dense)      round 3: attn(2) x out(1)
        round 1: attn(0) x qkv(2)            round 4: attn(3) x out(0b)+out(2)
        round 2: attn(1) x out(0a)+qkv(3)    round 5: out(3)
      (out(0) split at a token-tile boundary: 2 chains lead round 2 to
      hide the x-block-3 DMA, 2 reinforce the filler-starved round 4;
      outproj evacuates PSUM on ACT — a DVE evacuation A/B-measured
      21us SLOWER, it collides with the norm->ctx critical path)
  (c) attention heads in interleaved pairs (st x4 + o x2 + l x2 = 8
      PSUM banks), exp one tile behind S and L/PV two tiles behind, so
      ACT latency and both semaphore hops are off the PE critical path;
  (d) x blocks 0,1 live in persistent buffers refilled mid-iteration
      for the NEXT hw-loop iteration (xT is loop-invariant), block 2/3
      share the third buffer; kills the top-of-iteration DMA stall.

Steady state: sim ~313 us/iter (PE busy 307 us, gaps ~3 us); HW 403-418
us measured (thermal state shifts medians; the chip heats over ~1 s of
sustained work).  Baseline was 494 us; same-session A/B vs baseline
measured -55 us and later absolute runs reached 403-418 us.
"""

import contextlib
import math
import os

import numpy as np
import ml_dtypes

import bass_rust
import concourse.bass as bass
import concourse.mybir as mybir
import concourse.tile as tile
from concourse.bass import ts
from concourse.bass_utils import run_bass_kernel_spmd

BF16 = ml_dtypes.bfloat16
F32 = mybir.dt.float32
BF = mybir.dt.bfloat16

B = 2
S = 2048
D = 2048
HD = 128                    # head dim
NH = 16                     # total heads
NHL = 4                     # heads per core
FQ = NHL * HD               # 512 per-core q/k/v features
KC = D // 128               # 16 contraction chunks
TB = 4                      # token blocks of 512 (qkv phase)
IB = 4                      # query blocks of 512 (attention phase)
JT = S // 128               # 16 key tiles of 128
SCALE = 1.0 / math.sqrt(HD)
NEG = -30000.0              # additive mask; exp(NEG * SCALE) == 0 in fp32

MAX_WAITS = 1               # this walrus build allows 1 sync-wait per inst

_wait_ctr = [0]


def _split_all_multi_waits(nc):
    """This walrus build rejects instructions with >1 semaphore wait
    ("Too many sync wait commands").  Move extra waits onto NoOps inserted
    right before the instruction on the same engine (sequencers execute in
    order, so blocking one instruction earlier is equivalent)."""
    n_split = 0
    for f in nc.m.functions:
        for blk in f.blocks:
            out = []
            for inst in blk.instructions:
                si = inst.sync_info
                if si is not None and len(si.on_wait) > MAX_WAITS:
                    waits = list(si.on_wait)
                    for w in waits[:-MAX_WAITS]:
                        _wait_ctr[0] += 1
                        nop = mybir.InstNoOp(
                            name=f"I-waitsplit-{_wait_ctr[0]}", ins=[], outs=[]
                        )
                        nop.engine = inst.engine
                        nop.sync_info = bass_rust.SyncInfo(on_wait=[w], on_update=[])
                        out.append(nop)
                    inst.sync_info = bass_rust.SyncInfo(
                        on_wait=waits[-MAX_WAITS:], on_update=list(si.on_update)
                    )
                    n_split += 1
                out.append(inst)
            blk.instructions = out
    return n_split


def build_nc(
    reps: int = 1,
    split_waits: bool = True,
    loop: int = 1,
    do_qkv: bool = True,
    do_attn: bool = True,
    do_outproj: bool = True,
    dma_only: bool = False,
    skip_outdma: bool = False,
    split_outdma: bool = False,
    out_evac_dve: bool = False,
):
    nc = bass.Bass()
    xT = nc.declare_dram_parameter("xT", [128, TB, KC, 512], BF, isOutput=False)
    wqT = nc.declare_dram_parameter("wqT", [128, KC, FQ], BF, isOutput=False)
    wkT = nc.declare_dram_parameter("wkT", [128, KC, FQ], BF, isOutput=False)
    wvT = nc.declare_dram_parameter("wvT", [128, KC, FQ], BF, isOutput=False)
    woT = nc.declare_dram_parameter("woT", [128, NHL, D], BF, isOutput=False)
    cosT = nc.declare_dram_parameter("cosT", [128, S], BF, isOutput=False)
    sinT = nc.declare_dram_parameter("sinT", [128, S], BF, isOutput=False)
    maskd = nc.declare_dram_parameter("maskd", [128, 128], F32, isOutput=False)
    out = nc.declare_dram_parameter("out", [JT, 128, D], BF, isOutput=True)

    mult = mybir.AluOpType.mult
    add = mybir.AluOpType.add
    EXP = mybir.ActivationFunctionType.Exp

    with tile.TileContext(nc) as tc:
        with (
            tc.tile_pool(name="persist", bufs=1) as persist,
            tc.tile_pool(name="stream", bufs=1) as stream,
            tc.tile_pool(name="pp", bufs=4, space="PSUM") as pp,
        ):
            qT = persist.tile([128, NHL, S], BF, tag="qT")
            kT = persist.tile([128, NHL, S], BF, tag="kT")
            vv = persist.tile([128, JT, FQ], BF, tag="vv")   # [t-part, tt, d]
            ctx = persist.tile([128, NHL, S], BF, tag="ctx")
            cos_sb = persist.tile([128, S], BF, tag="cos")
            sin_sb = persist.tile([128, S], BF, tag="sin")
            # three persistent x-block buffers: A=block0, B=block1,
            # C=blocks 2 then 3 (reloaded mid-iteration)
            xs_bufs = [
                persist.tile(
                    [128, KC, 512], BF, tag=f"xs{i}", name=f"xs{i}"
                )
                for i in range(3)
            ]
            mask_sb = persist.tile([128, 128], F32, tag="mask")
            ones_sb = persist.tile([128, 128], BF, tag="ones")
            wq_sb = persist.tile([128, KC, FQ], BF, tag="wq")
            wk_sb = persist.tile([128, KC, FQ], BF, tag="wk")
            wv_sb = persist.tile([128, KC, FQ], BF, tag="wv")
            wo_sb = persist.tile([128, NHL, D], BF, tag="wo")

            nc.sync.dma_start(cos_sb[:], cosT[:])
            nc.sync.dma_start(sin_sb[:], sinT[:])
            nc.sync.dma_start(mask_sb[:], maskd[:])
            nc.vector.memset(ones_sb[:], 1.0)
            nc.sync.dma_start(wq_sb[:], wqT[:])
            nc.sync.dma_start(wk_sb[:], wkT[:])
            nc.sync.dma_start(wv_sb[:], wvT[:])
            nc.sync.dma_start(wo_sb[:], woT[:])

            def stage_tile():
                return stream.tile(
                    [128, D], BF, tag="stage", bufs=2, name="stage"
                )

            def qkv_thunks(tb, xs):
                """Fine-grained thunks for token block tb: 12 chains
                (4 v + 4 q + 4 k), each split into 4 sub-thunks of 4
                accumulating matmuls plus an evacuation thunk."""
                tbs = ts(tb, 512)
                thunks = []

                def qk_chain(w_sb, dstT, f):
                    box = {}

                    def mm4(g):
                        def run():
                            if g == 0:
                                box["ps"] = pp.tile(
                                    [128, 512], F32, tag="st", name="ps"
                                )
                            ps = box["ps"]
                            for kc in range(4 * g, 4 * g + 4):
                                nc.tensor.matmul(
                                    ps[:],
                                    w_sb[:, kc, ts(f, 128)],
                                    xs[:, kc, :],
                                    start=(kc == 0),
                                    stop=(kc == KC - 1),
                                )
                        return run

                    def evac():
                        # rope: dst = ps*cos + swap(ps)*sin_signed
                        ps = box["ps"]
                        t1 = stream.tile([128, 512], BF, tag="t1", bufs=2)
                        nc.vector.tensor_tensor(
                            t1[:], ps[:], cos_sb[:, tbs], mult
                        )
                        t2 = stream.tile([128, 512], BF, tag="t2", bufs=2)
                        nc.vector.tensor_tensor(
                            t2[0:64, :], ps[64:128, :], sin_sb[0:64, tbs], mult
                        )
                        nc.vector.tensor_tensor(
                            t2[64:128, :], ps[0:64, :], sin_sb[64:128, tbs],
                            mult,
                        )
                        nc.vector.tensor_tensor(
                            dstT[:, f, tbs], t1[:], t2[:], add
                        )

                    return [mm4(g) for g in range(4)] + [evac]

                def v_chain(s4):
                    box = {}

                    def mm4(g):
                        def run():
                            if g == 0:
                                box["ps"] = pp.tile(
                                    [128, 512], F32, tag="st", name="ps"
                                )
                            ps = box["ps"]
                            for kc in range(4 * g, 4 * g + 4):
                                nc.tensor.matmul(
                                    ps[:],
                                    xs[:, kc, ts(s4, 128)],
                                    wv_sb[:, kc, :],
                                    start=(kc == 0),
                                    stop=(kc == KC - 1),
                                )
                        return run

                    def evac():
                        nc.scalar.copy(vv[:, tb * 4 + s4, :], box["ps"][:])

                    return [mm4(g) for g in range(4)] + [evac]

                # k/q chains for low heads first so attention on this
                # block can start as early as possible; v interleaved
                order = [
                    ("k", 0), ("q", 0), ("k", 1), ("q", 1),
                    ("v", 0), ("v", 1),
                    ("k", 2), ("q", 2), ("k", 3), ("q", 3),
                    ("v", 2), ("v", 3),
                ]
                for kind, idx in order:
                    if kind == "v":
                        thunks.extend(v_chain(idx))
                    elif kind == "q":
                        thunks.extend(qk_chain(wq_sb, qT, idx))
                    else:
                        thunks.extend(qk_chain(wk_sb, kT, idx))
                return thunks

            def attn_thunks(ib):
                """Step-thunks: flash attention for query block ib.  Heads
                run in interleaved PAIRS so each head's exp latency hides
                behind the other head's S/L/PV matmuls.  Diagonal-crossing
                key tiles only compute columns [off:512]."""
                njt = 4 * ib + 4
                thunks = []

                def make_head(h):
                    sts = [None] * njt
                    pts = [None] * njt
                    acc = {}

                    def off_of(jt):
                        o = (jt - 4 * ib) * 128
                        return o if o > 0 else 0

                    def start_head():
                        acc["o"] = pp.tile(
                            [128, 512], F32, tag="o", bufs=2, name="o"
                        )
                        acc["l"] = pp.tile(
                            [128, 512], F32, tag="l", bufs=2, name="l"
                        )

                    def emit_s(jt):
                        off = off_of(jt)
                        st = pp.tile([128, 512], F32, tag="st", name="st")
                        nc.tensor.matmul(
                            st[:, off:512],
                            kT[:, h, ts(jt, 128)],
                            qT[:, h, ib * 512 + off : ib * 512 + 512],
                            start=True,
                            stop=True,
                        )
                        sts[jt] = st

                    def emit_exp(jt):
                        st = sts[jt]
                        off = off_of(jt)
                        pt = stream.tile([128, 512], BF, tag="pt", bufs=6)
                        if jt >= 4 * ib:
                            nc.vector.tensor_tensor(
                                st[:, off : off + 128],
                                st[:, off : off + 128],
                                mask_sb[:],
                                add,
                            )
                        nc.scalar.activation(
                            pt[:, off:512], st[:, off:512], EXP, scale=SCALE
                        )
                        pts[jt] = pt

                    def emit_l(jt):
                        off = off_of(jt)
                        nc.tensor.matmul(
                            acc["l"][:, off:512], ones_sb[:],
                            pts[jt][:, off:512],
                            start=(jt == 0), stop=(jt == njt - 1),
                        )

                    def emit_pv(jt):
                        off = off_of(jt)
                        nc.tensor.matmul(
                            acc["o"][:, off:512], vv[:, jt, ts(h, 128)],
                            pts[jt][:, off:512],
                            start=(jt == 0), stop=(jt == njt - 1),
                        )

                    def norm_head():
                        linv = stream.tile([128, 512], F32, tag="linv", bufs=2)
                        nc.vector.reciprocal(linv[:], acc["l"][:])
                        nc.vector.tensor_tensor(
                            ctx[:, h, ts(ib, 512)], acc["o"][:], linv[:], mult
                        )

                    return start_head, emit_s, emit_exp, emit_l, emit_pv, \
                        norm_head

                for hp in (0, 2):
                    s0, es0, ex0, el0, ep0, n0 = make_head(hp)
                    s1, es1, ex1, el1, ep1, n1 = make_head(hp + 1)
                    thunks.append(s0)
                    thunks.append(s1)
                    for jt in range(njt + 2):
                        if jt < njt:
                            thunks.append(lambda jt=jt, f=es0: f(jt))
                            thunks.append(lambda jt=jt, f=es1: f(jt))
                        if 1 <= jt <= njt:
                            # exp one tile behind S
                            thunks.append(lambda jt=jt - 1, f=ex0: f(jt))
                            thunks.append(lambda jt=jt - 1, f=ex1: f(jt))
                        if jt >= 2:
                            # L/PV two tiles behind S: a full round of slack
                            # for the ACT exp and its semaphore hops
                            thunks.append(lambda jt=jt - 2, f=el0: f(jt))
                            thunks.append(lambda jt=jt - 2, f=el1: f(jt))
                            thunks.append(lambda jt=jt - 2, f=ep0: f(jt))
                            thunks.append(lambda jt=jt - 2, f=ep1: f(jt))
                    thunks.append(n0)
                    thunks.append(n1)
                return thunks

            def outproj_thunks(ib):
                """Fine-grained: per token sub-block tt, 4 (ob) thunks of a
                4-matmul accumulation chain + ACT evacuation, then a DMA."""
                thunks = []

                def one(tt):
                    box = {}

                    def ob_thunk(ob):
                        def run():
                            if ob == 0:
                                box["stage"] = stage_tile()
                            ps = pp.tile([128, 512], F32, tag="st", name="ps")
                            for fc in range(NHL):
                                nc.tensor.matmul(
                                    ps[:],
                                    ctx[:, fc, ts(tt, 128)],
                                    wo_sb[:, fc, ts(ob, 512)],
                                    start=(fc == 0),
                                    stop=(fc == NHL - 1),
                                )
                            if out_evac_dve:
                                # DVE evacuation: keeps the ACT FIFO free
                                # for exps in the attention rounds and
                                # releases the PSUM bank sooner
                                nc.vector.tensor_scalar_add(
                                    box["stage"][:, ts(ob, 512)], ps[:], 0.0
                                )
                            else:
                                nc.scalar.copy(
                                    box["stage"][:, ts(ob, 512)], ps[:]
                                )
                            if split_outdma and not skip_outdma:
                                nc.sync.dma_start(
                                    out[tt, :, ts(ob, 512)],
                                    box["stage"][:, ts(ob, 512)],
                                )
                        return run

                    def dma():
                        if not skip_outdma:
                            nc.sync.dma_start(out[tt], box["stage"][:])

                    thl = [ob_thunk(ob) for ob in range(4)]
                    if not split_outdma:
                        thl.append(dma)
                    return thl

                for tt in range(4 * ib, 4 * ib + 4):
                    thunks.extend(one(tt))
                return thunks

            def run_interleaved(big, small):
                """Emit `small` steps spread evenly between `big` steps."""
                n, m = len(big), len(small)
                j = 0
                for i, b in enumerate(big):
                    b()
                    take = (m * (i + 1)) // n - j
                    for _ in range(take):
                        small[j]()
                        j += 1
                while j < m:
                    small[j]()
                    j += 1

            # prologue (outside the hw loop): blocks 0,1 for iteration 0
            if do_qkv and not dma_only:
                nc.sync.dma_start(xs_bufs[0][:], xT[:, 0])
                nc.sync.dma_start(xs_bufs[1][:], xT[:, 1])

            loop_cm = tc.For_i(0, loop, 1) if loop > 1 else contextlib.nullcontext()
            with loop_cm:
                for _rep in range(reps):
                    if dma_only:
                        for tb in range(TB):
                            nc.sync.dma_start(xs_bufs[tb % 3][:], xT[:, tb])
                        stage0 = stage_tile()
                        nc.vector.memset(stage0[:], 0.0)
                        for tt in range(JT):
                            nc.sync.dma_start(out[tt], stage0[:])
                        continue

                    if not do_qkv:
                        nc.vector.memset(qT[:], 0.0)
                        nc.vector.memset(kT[:], 0.0)
                        nc.vector.memset(vv[:], 0.0)
                    if not do_attn:
                        nc.vector.memset(ctx[:], 0.0)

                    A = (lambda ib: attn_thunks(ib)) if do_attn else (
                        lambda ib: []
                    )
                    O = (lambda ib: outproj_thunks(ib)) if do_outproj else (
                        lambda ib: []
                    )

                    if do_qkv:
                        xsA, xsB, xsC = xs_bufs
                        # blocks 0,1 already resident (prologue / previous
                        # iteration tail).  Load block 2 now.
                        nc.sync.dma_start(xsC[:], xT[:, 2])
                        # round 0: qkv blocks 0,1 dense
                        for t in qkv_thunks(0, xsA):
                            t()
                        for t in qkv_thunks(1, xsB):
                            t()
                        # round 1: attn(0) x qkv(2)
                        run_interleaved(A(0), qkv_thunks(2, xsC))
                        # block 3 reuses C (waits for qkv(2) reads)
                        nc.sync.dma_start(xsC[:], xT[:, 3])
                        # round 2: attn(1) x out(0)[:2 chains] + qkv(3);
                        # out(0) leads so its matmuls hide the block-3 DMA
                        o0 = O(0)
                        run_interleaved(A(1), o0[:10] + qkv_thunks(3, xsC))
                        # refill blocks 0,1 for the next hw-loop iteration
                        # (xT is loop-invariant, so this is idempotent)
                        nc.sync.dma_start(xsA[:], xT[:, 0])
                        nc.sync.dma_start(xsB[:], xT[:, 1])
                        # round 3: attn(2) x out(1)
                        run_interleaved(A(2), O(1))
                        # round 4: attn(3) x out(0)[2:] + out(2)
                        run_interleaved(A(3), o0[10:] + O(2))
                        # round 5: out(3)
                        for t in O(3):
                            t()
                    else:
                        for ib in range(IB):
                            for t in A(ib):
                                t()
                            for t in O(ib):
                                t()

    if split_waits:
        _split_all_multi_waits(nc)
    return nc


def _rope_tables():
    inv_freq = 1.0 / (10000.0 ** (np.arange(0, HD, 2, dtype=np.float32) / HD))
    t = np.arange(S, dtype=np.float32)
    freqs = np.einsum("i,j->ij", t, inv_freq)          # [S, 64]
    emb = np.concatenate([freqs, freqs], axis=-1)      # [S, 128]
    cos = np.cos(emb).T.astype(np.float32)             # [128, S]
    sin = np.sin(emb).T.astype(np.float32)             # [128, S]
    sin_signed = sin.copy()
    sin_signed[:64] *= -1.0                            # rotate_half sign fold
    return np.ascontiguousarray(cos), np.ascontiguousarray(sin_signed)


def _mask_diag():
    jj = np.arange(128)[:, None]
    ii = np.arange(128)[None, :]
    return np.where(ii >= jj, 0.0, NEG).astype(np.float32)


def _chunk_pmajor(a):
    """[R, C] with R = n*128 -> [128, n, C] with out[p, n, c] = a[n*128+p, c]."""
    n = a.shape[0] // 128
    return np.ascontiguousarray(a.reshape(n, 128, -1).transpose(1, 0, 2))


def make_in_maps(x, w_qkv, w_out):
    cos, sin_signed = _rope_tables()
    mask = _mask_diag()
    in_maps = []
    xT_by_b = []
    for b in range(B):
        # xT[p, tb, kc, t'] = x[b, tb*512+t', kc*128+p]
        xt = _chunk_pmajor(x[b].T.astype(np.float32))          # [128, KC, S]
        xt = xt.reshape(128, KC, TB, 512).transpose(0, 2, 1, 3)
        xT_by_b.append(np.ascontiguousarray(xt).astype(BF16))
    for c in range(8):
        b, hg = c // 4, c % 4
        rows = slice(hg * FQ, (hg + 1) * FQ)
        wq = _chunk_pmajor(w_qkv[0 * D:][rows].T).astype(BF16)   # [128, KC, FQ]
        wk = _chunk_pmajor(w_qkv[1 * D:][rows].T).astype(BF16)
        wv = _chunk_pmajor(w_qkv[2 * D:][rows].T).astype(BF16)
        wo = _chunk_pmajor(w_out[:, hg * FQ:(hg + 1) * FQ].T).astype(BF16)
        in_maps.append(
            {
                "xT": xT_by_b[b],
                "wqT": wq,
                "wkT": wk,
                "wvT": wv,
                "woT": wo,
                "cosT": cos.astype(BF16),
                "sinT": sin_signed.astype(BF16),
                "maskd": mask,
            }
        )
    return in_maps


_nc_cache = {}


def kernel(x, w_qkv, w_out):
    x = np.asarray(x)
    w_qkv = np.asarray(w_qkv)
    w_out = np.asarray(w_out)
    reps = int(os.environ.get("KERNEL_REPS", "1"))
    if reps not in _nc_cache:
        _nc_cache[reps] = build_nc(reps)
    nc = _nc_cache[reps]
    in_maps = make_in_maps(x, w_qkv, w_out)
    res = run_bass_kernel_spmd(nc, in_maps, list(range(8)), trace=False)
    out = np.zeros((B, S, D), dtype=np.float32)
    for c in range(8):
        out[c // 4] += res.results[c]["out"].reshape(S, D).astype(np.float32)
    return out
